# revision 27
# baseline (speedup 1.0000x reference)
"""Trainium2 Bass kernel for a dense multi-head attention layer.

Problem (full shapes): query/key/value [4, 2048, 1024] fp32, Wq/Wk/Wv
[1024, 1024], bq/bk/bv [1024].  out = MHA(q,k,v) with H=16 heads of 64.

Sharding over 8 NeuronCores: core c handles batch c//2 and head-half c%2
(8 heads = 512 of the 1024 output dims).  Each core runs an identical
program on its shard; the host assembles the full [4, 2048, 1024] output.

Per-core pipeline (all matmuls in float32r = full-rate fp32 storage):
  1. PE-transpose W slices -> WT [c, d] and x tiles -> xT [c, t].
  2. Projections: kT/qT [d, t] (stored bf16; scores are scaled by 1/8 so
     bf16 rounding of q/k is harmless) and v [t, d] fp32 with a ones
     column appended per head (65-wide blocks).
  3. Attention per head-pair/i-block: scores computed transposed
     [j, i] = kT.T @ qT, exp on ScalarE directly from PSUM with the
     1/sqrt(64) scale folded in (no max-subtraction: scores are O(6)),
     then ctx.T [hd+1, i] = v_aug.T @ exp accumulated over j; row 64 is
     the softmax denominator for free.
  4. Finalize: PE-transpose [65, 128] chunks -> [t, hd+1], reciprocal of
     the denominator column, scale, DMA out.
"""

import numpy as np

_NC_CACHE = {}

S = 2048  # tokens per batch
C = 1024  # model dim (contraction)
DH = 512  # output dims per core (8 heads x 64)
NHEAD = 8  # heads per core
HD = 64  # head dim
NCORES = 8
TB = 512  # token block
NTB = S // TB  # 4
NCC = C // 128  # 8 c-chunks
NDD = DH // 128  # 4 d-tiles (= head pairs)
NTT = S // 128  # 16 t-tiles
SCALE = 0.125  # 1/sqrt(64)


def _build_module():
    import concourse.mybir as mybir
    import concourse.tile as tile
    from concourse import bacc
    from concourse.bass import ds, ts
    from concourse.masks import make_identity

    F32 = mybir.dt.float32
    F32R = mybir.dt.float32r
    BF16 = mybir.dt.bfloat16
    EXP = mybir.ActivationFunctionType.Exp

    nc = bacc.Bacc("TRN2", target_bir_lowering=False)

    xq = nc.dram_tensor("xq", [S, C], F32R, kind="ExternalInput")
    xk = nc.dram_tensor("xk", [S, C], F32R, kind="ExternalInput")
    xv = nc.dram_tensor("xv", [S, C], F32R, kind="ExternalInput")
    wq = nc.dram_tensor("wq", [DH, C], F32R, kind="ExternalInput")
    wk = nc.dram_tensor("wk", [DH, C], F32R, kind="ExternalInput")
    wv = nc.dram_tensor("wv", [DH, C], F32R, kind="ExternalInput")
    bq = nc.dram_tensor("bq", [DH], F32, kind="ExternalInput")
    bk = nc.dram_tensor("bk", [DH], F32, kind="ExternalInput")
    bv = nc.dram_tensor("bv", [DH], F32, kind="ExternalInput")
    out = nc.dram_tensor("out", [S, DH], F32, kind="ExternalOutput")

    with tile.TileContext(nc) as tc:
        from contextlib import ExitStack

        with ExitStack() as ctx:
            consts = ctx.enter_context(tc.tile_pool(name="consts", bufs=1))
            ld = ctx.enter_context(tc.tile_pool(name="ld", bufs=10))
            xt = ctx.enter_context(tc.tile_pool(name="xt", bufs=16))
            wt = ctx.enter_context(tc.tile_pool(name="wt", bufs=1))
            qk = ctx.enter_context(tc.tile_pool(name="qk", bufs=1))
            vp = ctx.enter_context(tc.tile_pool(name="vp", bufs=1))
            cs = ctx.enter_context(tc.tile_pool(name="cs", bufs=2))
            xp = ctx.enter_context(tc.tile_pool(name="xp", bufs=6))
            outp = ctx.enter_context(tc.tile_pool(name="outp", bufs=6))
            tiny = ctx.enter_context(tc.tile_pool(name="tiny", bufs=6))
            ps_mm = ctx.enter_context(
                tc.tile_pool(name="ps_mm", bufs=2, space="PSUM")
            )
            ps_ctx = ctx.enter_context(
                tc.tile_pool(name="ps_ctx", bufs=1, space="PSUM")
            )
            ps_misc = ctx.enter_context(
                tc.tile_pool(name="ps_misc", bufs=2, space="PSUM")
            )

            ident_f32 = consts.tile([128, 128], F32, tag="ident_f32")
            make_identity(nc, ident_f32)
            identity = consts.tile([128, 128], F32R, tag="ident")
            nc.vector.tensor_copy(identity, ident_f32)
            identity_bf = consts.tile([128, 128], BF16, tag="identb")
            nc.vector.tensor_copy(identity_bf, ident_f32)
            ones8 = consts.tile([128, 8], F32, tag="ones8")
            nc.vector.memset(ones8, 1.0)
            zc = consts.tile([128, 2 * TB], F32, tag="zc")
            nc.vector.memset(zc, 0.0)
            # Load the exp table during phase 1 (first ACTIVATE pays ~2.7us).
            warm = consts.tile([128, 1], F32, tag="warm")
            nc.scalar.activation(warm, ones8[:, 0:1], EXP)

            # Per-partition bias columns for qT/kT [d on partitions]:
            # col a holds bias[a*128 + p].
            bqt = consts.tile([128, NDD], F32, tag="bqt")
            nc.sync.dma_start(out=bqt, in_=bq[:].rearrange("(a p) -> p a", p=128))
            bkt = consts.tile([128, NDD], F32, tag="bkt")
            nc.sync.dma_start(out=bkt, in_=bk[:].rearrange("(a p) -> p a", p=128))
            # v bias broadcast along token partitions.
            bvb = consts.tile([128, DH], F32, tag="bvb")
            nc.sync.dma_start(
                out=bvb,
                in_=bv[:].rearrange("(o c) -> o c", o=1).to_broadcast([128, DH]),
            )

            # ---- weight transposes: W [d, c] -> WT [c, (cc, d)] ----
            # Cast fp32 -> bf16 during the load (SWDGE), then transpose
            # 128x128 blocks on the DMA xbar engine (HWDGE) -- zero PE work.
            wts = {}
            for name, wsrc in (("q", wq), ("k", wk), ("v", wv)):
                wtile = wt.tile([128, NCC, DH], BF16, tag=f"wt_{name}")
                wts[name] = wtile
                lds = []
                for p in range(NDD):
                    t = ld.tile([128, C], BF16, tag="ld")
                    nc.gpsimd.dma_start(out=t, in_=wsrc[ts(p, 128), :])
                    lds.append(t)
                for cc in range(NCC):
                    ps = ps_misc.tile([128, DH], BF16, tag="misc")
                    for p in range(NDD):
                        nc.tensor.transpose(
                            ps[:, ts(p, 128)],
                            lds[p][:, ts(cc, 128)],
                            identity_bf,
                        )
                    nc.vector.tensor_copy(wtile[:, cc, :], ps)

            # ---- projections, k and v first, then q ----
            # qkT[(name, dd, T)] : [128 d, 512 t] bf16
            qkt = {}
            # v_sb[tt] : [128 t, 8*65] fp32, ones in col 64 of each 65-block
            v_sb = []
            for tt in range(NTT):
                vt = vp.tile([128, NHEAD * 65], BF16, tag=f"v{tt}")
                v_sb.append(vt)
                ones_view = vt.rearrange("p (h c) -> p h c", c=65)[:, :, 64:65]
                nc.vector.tensor_copy(
                    ones_view, ones8.rearrange("p (h o) -> p h o", o=1)
                )

            def xpose_block(xsrc, T):
                """Load x[T*512:(T+1)*512] (cast to bf16) and return 8 xT
                tiles [128 c, 512 t] produced by xbar DMA transpose."""
                lds = []
                for i in range(4):
                    t = ld.tile([128, C], BF16, tag="ld")
                    nc.gpsimd.dma_start(out=t, in_=xsrc[ts(4 * T + i, 128), :])
                    lds.append(t)
                xts = []
                for cc in range(NCC):
                    ps = ps_misc.tile([128, TB], BF16, tag="misc")
                    for i in range(4):
                        nc.tensor.transpose(
                            ps[:, ts(i, 128)],
                            lds[i][:, ts(cc, 128)],
                            identity_bf,
                        )
                    xtile = xt.tile([128, TB], BF16, tag="xt")
                    nc.vector.tensor_copy(xtile, ps)
                    xts.append(xtile)
                return xts

            def proj_qk(name, xts, btile, T):
                for dd in range(NDD):
                    psq = ps_mm.tile([128, 2 * TB], F32, tag="mm")
                    for cc in range(NCC):
                        nc.tensor.matmul(
                            psq[:, 0:TB],
                            wts[name][:, cc, ts(dd, 128)],
                            xts[cc],
                            start=(cc == 0),
                            stop=(cc == NCC - 1),
                        )
                    qt = qk.tile([128, TB], BF16, tag=f"{name}{dd}{T}")
                    qkt[(name, dd, T)] = qt
                    nc.vector.tensor_scalar_add(
                        qt, psq[:, 0:TB], btile[:, dd : dd + 1]
                    )

            def attention_unit(hp, I):
                hA, hB = 2 * hp, 2 * hp + 1
                qtile = qkt[("q", hp, I)]
                ctx_ps = ps_ctx.tile([65, 2 * TB], F32, tag="ctx")
                for J in range(NTT):  # j tile of 128 keys
                    ktile = qkt[("k", hp, J // 4)]
                    jj = (J % 4) * 128
                    sc = ps_mm.tile([128, 2 * TB], F32, tag="mm")
                    nc.tensor.matmul(
                        sc[:, 0:TB],
                        ktile[0:64, jj : jj + 128],
                        qtile[0:64, :],
                    )
                    nc.tensor.matmul(
                        sc[:, TB : 2 * TB],
                        ktile[64:128, jj : jj + 128],
                        qtile[64:128, :],
                    )
                    ex = xp.tile([128, 2 * TB], BF16, tag="exp")
                    nc.scalar.activation(ex, sc, EXP, scale=SCALE)
                    nc.tensor.matmul(
                        ctx_ps[:, 0:TB],
                        v_sb[J][:, 65 * hA : 65 * hA + 65],
                        ex[:, 0:TB],
                        start=(J == 0),
                        stop=(J == NTT - 1),
                    )
                    nc.tensor.matmul(
                        ctx_ps[:, TB : 2 * TB],
                        v_sb[J][:, 65 * hB : 65 * hB + 65],
                        ex[:, TB : 2 * TB],
                        start=(J == 0),
                        stop=(J == NTT - 1),
                    )
                # transpose contraction dim must be a multiple of 32:
                # pad [65, .] to [96, .] with zeros in rows 65:96.
                ctx_sb = cs.tile([96, 2 * TB], F32R, tag="ctxsb")
                nc.vector.tensor_copy(ctx_sb[64:96, :], zc[64:96, :])
                nc.vector.tensor_copy(ctx_sb[0:65, :], ctx_ps)
                for q in range(4):  # 128-token output tiles within I
                    st = outp.tile([128, 128], F32, tag="st")
                    for h2 in range(2):
                        off = h2 * TB + q * 128
                        fin = ps_misc.tile([128, 96], F32R, tag="misc")
                        nc.tensor.transpose(
                            fin,
                            ctx_sb[:, off : off + 128],
                            identity[0:96, 0:96],
                        )
                        rec = tiny.tile([128, 1], F32, tag="rec")
                        nc.vector.reciprocal(rec, fin[:, 64:65])
                        nc.vector.tensor_scalar_mul(
                            st[:, 64 * h2 : 64 * h2 + 64], fin[:, 0:64], rec
                        )
                    nc.sync.dma_start(
                        out=out[ds(I * TB + q * 128, 128), ds(hp * 128, 128)],
                        in_=st,
                    )

            # k and v first (attention consumes both in full), then per
            # q-block start the attention units for that i-block.
            for T in range(NTB):
                xts = xpose_block(xk, T)
                proj_qk("k", xts, bkt, T)
            for T in range(NTB):
                xts = xpose_block(xv, T)
                for i in range(4):
                    tt = 4 * T + i
                    psv = ps_mm.tile([128, 2 * TB], F32, tag="mm")
                    for cc in range(NCC):
                        nc.tensor.matmul(
                            psv[:, 0:DH],
                            xts[cc][:, ts(i, 128)],
                            wts["v"][:, cc, :],
                            start=(cc == 0),
                            stop=(cc == NCC - 1),
                        )
                    for h in range(NHEAD):
                        nc.vector.tensor_add(
                            v_sb[tt][:, 65 * h : 65 * h + 64],
                            psv[:, 64 * h : 64 * h + 64],
                            bvb[:, 64 * h : 64 * h + 64],
                        )
            for T in range(NTB):
                xts = xpose_block(xq, T)
                proj_qk("q", xts, bqt, T)
                for hp in range(NDD):
                    attention_unit(hp, I=T)
    nc.finalize()
    return nc


def _get_module():
    if "nc" not in _NC_CACHE:
        _NC_CACHE["nc"] = _build_module()
    return _NC_CACHE["nc"]


def kernel(**inputs) -> np.ndarray:
    from concourse.bass_utils import run_bass_kernel_spmd

    query = np.asarray(inputs["query"], dtype=np.float32)
    key = np.asarray(inputs["key"], dtype=np.float32)
    value = np.asarray(inputs["value"], dtype=np.float32)
    Wq = np.asarray(inputs["Wq"], dtype=np.float32)
    Wk = np.asarray(inputs["Wk"], dtype=np.float32)
    Wv = np.asarray(inputs["Wv"], dtype=np.float32)
    bq = np.asarray(inputs["bq"], dtype=np.float32)
    bk = np.asarray(inputs["bk"], dtype=np.float32)
    bv = np.asarray(inputs["bv"], dtype=np.float32)

    B = query.shape[0]
    nc = _get_module()
    in_maps = []
    for c in range(NCORES):
        b, g = c // 2, c % 2
        sl = slice(g * DH, (g + 1) * DH)
        in_maps.append(
            {
                "xq": np.ascontiguousarray(query[b]),
                "xk": np.ascontiguousarray(key[b]),
                "xv": np.ascontiguousarray(value[b]),
                "wq": np.ascontiguousarray(Wq[sl]),
                "wk": np.ascontiguousarray(Wk[sl]),
                "wv": np.ascontiguousarray(Wv[sl]),
                "bq": np.ascontiguousarray(bq[sl]),
                "bk": np.ascontiguousarray(bk[sl]),
                "bv": np.ascontiguousarray(bv[sl]),
            }
        )
    res = run_bass_kernel_spmd(nc, in_maps, core_ids=list(range(NCORES)))
    full = np.empty((B, S, C), dtype=np.float32)
    for c in range(NCORES):
        b, g = c // 2, c % 2
        full[b, :, g * DH : (g + 1) * DH] = res.results[c]["out"]
    return full


if __name__ == "__main__":
    import reference

    inputs = {k: np.asarray(v) for k, v in reference.setup_inputs().items()}
    got = kernel(**inputs)
    want = np.asarray(reference.reference(**reference.setup_inputs()))
    err = np.abs(got - want).max() / np.abs(want).max()
    print("rel err:", err)


# revision 28
# speedup vs baseline: 1.0356x; 1.0356x over previous
"""Trainium2 Bass kernel for a dense multi-head attention layer.

Problem (full shapes): query/key/value [4, 2048, 1024] fp32, Wq/Wk/Wv
[1024, 1024], bq/bk/bv [1024].  out = MHA(q,k,v) with H=16 heads of 64.

Sharding over 8 NeuronCores: core c handles batch c//2 and head-half c%2
(8 heads = 512 of the 1024 output dims).  Each core runs an identical
program on its shard; the host assembles the full [4, 2048, 1024] output.

Per-core pipeline (all matmuls in float32r = full-rate fp32 storage):
  1. PE-transpose W slices -> WT [c, d] and x tiles -> xT [c, t].
  2. Projections: kT/qT [d, t] (stored bf16; scores are scaled by 1/8 so
     bf16 rounding of q/k is harmless) and v [t, d] fp32 with a ones
     column appended per head (65-wide blocks).
  3. Attention per head-pair/i-block: scores computed transposed
     [j, i] = kT.T @ qT, exp on ScalarE directly from PSUM with the
     1/sqrt(64) scale folded in (no max-subtraction: scores are O(6)),
     then ctx.T [hd+1, i] = v_aug.T @ exp accumulated over j; row 64 is
     the softmax denominator for free.
  4. Finalize: PE-transpose [65, 128] chunks -> [t, hd+1], reciprocal of
     the denominator column, scale, DMA out.
"""

import numpy as np

_NC_CACHE = {}

S = 2048  # tokens per batch
C = 1024  # model dim (contraction)
DH = 512  # output dims per core (8 heads x 64)
NHEAD = 8  # heads per core
HD = 64  # head dim
NCORES = 8
TB = 512  # token block
NTB = S // TB  # 4
NCC = C // 128  # 8 c-chunks
NDD = DH // 128  # 4 d-tiles (= head pairs)
NTT = S // 128  # 16 t-tiles
SCALE = 0.125  # 1/sqrt(64)


def _build_module():
    import concourse.mybir as mybir
    import concourse.tile as tile
    from concourse import bacc
    from concourse.bass import ds, ts
    from concourse.masks import make_identity

    F32 = mybir.dt.float32
    F32R = mybir.dt.float32r
    BF16 = mybir.dt.bfloat16
    EXP = mybir.ActivationFunctionType.Exp

    nc = bacc.Bacc("TRN2", target_bir_lowering=False)

    xq = nc.dram_tensor("xq", [S, C], F32R, kind="ExternalInput")
    xk = nc.dram_tensor("xk", [S, C], F32R, kind="ExternalInput")
    xv = nc.dram_tensor("xv", [S, C], F32R, kind="ExternalInput")
    wq = nc.dram_tensor("wq", [DH, C], F32R, kind="ExternalInput")
    wk = nc.dram_tensor("wk", [DH, C], F32R, kind="ExternalInput")
    wv = nc.dram_tensor("wv", [DH, C], F32R, kind="ExternalInput")
    bq = nc.dram_tensor("bq", [DH], F32, kind="ExternalInput")
    bk = nc.dram_tensor("bk", [DH], F32, kind="ExternalInput")
    bv = nc.dram_tensor("bv", [DH], F32, kind="ExternalInput")
    out = nc.dram_tensor("out", [S, DH], F32, kind="ExternalOutput")

    with tile.TileContext(nc) as tc:
        from contextlib import ExitStack

        with ExitStack() as ctx:
            consts = ctx.enter_context(tc.tile_pool(name="consts", bufs=1))
            ld = ctx.enter_context(tc.tile_pool(name="ld", bufs=10))
            xt = ctx.enter_context(tc.tile_pool(name="xt", bufs=16))
            wt = ctx.enter_context(tc.tile_pool(name="wt", bufs=1))
            qk = ctx.enter_context(tc.tile_pool(name="qk", bufs=1))
            vp = ctx.enter_context(tc.tile_pool(name="vp", bufs=1))
            cs = ctx.enter_context(tc.tile_pool(name="cs", bufs=2))
            xp = ctx.enter_context(tc.tile_pool(name="xp", bufs=6))
            outp = ctx.enter_context(tc.tile_pool(name="outp", bufs=6))
            tiny = ctx.enter_context(tc.tile_pool(name="tiny", bufs=6))
            ps_mm = ctx.enter_context(
                tc.tile_pool(name="ps_mm", bufs=2, space="PSUM")
            )
            ps_ctx = ctx.enter_context(
                tc.tile_pool(name="ps_ctx", bufs=1, space="PSUM")
            )
            ps_misc = ctx.enter_context(
                tc.tile_pool(name="ps_misc", bufs=2, space="PSUM")
            )

            ident_f32 = consts.tile([128, 128], F32, tag="ident_f32")
            make_identity(nc, ident_f32)
            identity = consts.tile([128, 128], F32R, tag="ident")
            nc.vector.tensor_copy(identity, ident_f32)
            identity_bf = consts.tile([128, 128], BF16, tag="identb")
            nc.vector.tensor_copy(identity_bf, ident_f32)
            ones8 = consts.tile([128, 8], F32, tag="ones8")
            nc.vector.memset(ones8, 1.0)
            zc = consts.tile([128, 2 * TB], F32, tag="zc")
            nc.vector.memset(zc, 0.0)
            # Load the exp table during phase 1 (first ACTIVATE pays ~2.7us).
            warm = consts.tile([128, 1], F32, tag="warm")
            nc.scalar.activation(warm, ones8[:, 0:1], EXP)

            # Per-partition bias columns for qT/kT [d on partitions]:
            # col a holds bias[a*128 + p].
            bqt = consts.tile([128, NDD], F32, tag="bqt")
            nc.sync.dma_start(out=bqt, in_=bq[:].rearrange("(a p) -> p a", p=128))
            bkt = consts.tile([128, NDD], F32, tag="bkt")
            nc.sync.dma_start(out=bkt, in_=bk[:].rearrange("(a p) -> p a", p=128))
            # v bias broadcast along token partitions.
            bvb = consts.tile([128, DH], F32, tag="bvb")
            nc.sync.dma_start(
                out=bvb,
                in_=bv[:].rearrange("(o c) -> o c", o=1).to_broadcast([128, DH]),
            )

            # ---- weight transposes: W [d, c] -> WT [c, (cc, d)] ----
            # Cast fp32 -> bf16 during the load (SWDGE), then transpose
            # 128x128 blocks on the DMA xbar engine (HWDGE) -- zero PE work.
            wts = {}
            for name, wsrc in (("q", wq), ("k", wk), ("v", wv)):
                wtile = wt.tile([128, NCC, DH], BF16, tag=f"wt_{name}")
                wts[name] = wtile
                lds = []
                for p in range(NDD):
                    t = ld.tile([128, C], BF16, tag="ld")
                    nc.gpsimd.dma_start(out=t, in_=wsrc[ts(p, 128), :])
                    lds.append(t)
                for cc in range(NCC):
                    ps = ps_misc.tile([128, DH], BF16, tag="misc")
                    for p in range(NDD):
                        nc.tensor.transpose(
                            ps[:, ts(p, 128)],
                            lds[p][:, ts(cc, 128)],
                            identity_bf,
                        )
                    nc.vector.tensor_copy(wtile[:, cc, :], ps)

            # ---- projections, k and v first, then q ----
            # qkT[(name, dd, T)] : [128 d, 512 t] bf16
            qkt = {}
            # v_sb[tt] : [128 t, 8*65] fp32, ones in col 64 of each 65-block
            v_sb = []
            for tt in range(NTT):
                vt = vp.tile([128, NHEAD * 65], BF16, tag=f"v{tt}")
                v_sb.append(vt)
                ones_view = vt.rearrange("p (h c) -> p h c", c=65)[:, :, 64:65]
                nc.vector.tensor_copy(
                    ones_view, ones8.rearrange("p (h o) -> p h o", o=1)
                )

            def xpose_block(xsrc, T):
                """Load x[T*512:(T+1)*512] (cast to bf16) and return 8 xT
                tiles [128 c, 512 t] produced by xbar DMA transpose."""
                lds = []
                for i in range(4):
                    t = ld.tile([128, C], BF16, tag="ld")
                    nc.gpsimd.dma_start(out=t, in_=xsrc[ts(4 * T + i, 128), :])
                    lds.append(t)
                xts = []
                for cc in range(NCC):
                    ps = ps_misc.tile([128, TB], BF16, tag="misc")
                    for i in range(4):
                        nc.tensor.transpose(
                            ps[:, ts(i, 128)],
                            lds[i][:, ts(cc, 128)],
                            identity_bf,
                        )
                    xtile = xt.tile([128, TB], BF16, tag="xt")
                    nc.vector.tensor_copy(xtile, ps)
                    xts.append(xtile)
                return xts

            def proj_qk(name, xts, btile, T):
                for dd in range(NDD):
                    psq = ps_mm.tile([128, 2 * TB], F32, tag="mm")
                    for cc in range(NCC):
                        nc.tensor.matmul(
                            psq[:, 0:TB],
                            wts[name][:, cc, ts(dd, 128)],
                            xts[cc],
                            start=(cc == 0),
                            stop=(cc == NCC - 1),
                        )
                    qt = qk.tile([128, TB], BF16, tag=f"{name}{dd}{T}")
                    qkt[(name, dd, T)] = qt
                    nc.vector.tensor_scalar_add(
                        qt, psq[:, 0:TB], btile[:, dd : dd + 1]
                    )

            def attention_unit(hp, I):
                hA, hB = 2 * hp, 2 * hp + 1
                qtile = qkt[("q", hp, I)]
                ctx_ps = ps_ctx.tile([65, 2 * TB], F32, tag="ctx")
                for J in range(NTT):  # j tile of 128 keys
                    ktile = qkt[("k", hp, J // 4)]
                    jj = (J % 4) * 128
                    sc = ps_mm.tile([128, 2 * TB], F32, tag="mm")
                    nc.tensor.matmul(
                        sc[:, 0:TB],
                        ktile[0:64, jj : jj + 128],
                        qtile[0:64, :],
                    )
                    nc.tensor.matmul(
                        sc[:, TB : 2 * TB],
                        ktile[64:128, jj : jj + 128],
                        qtile[64:128, :],
                    )
                    ex = xp.tile([128, 2 * TB], BF16, tag="exp")
                    nc.scalar.activation(ex, sc, EXP, scale=SCALE)
                    nc.tensor.matmul(
                        ctx_ps[:, 0:TB],
                        v_sb[J][:, 65 * hA : 65 * hA + 65],
                        ex[:, 0:TB],
                        start=(J == 0),
                        stop=(J == NTT - 1),
                    )
                    nc.tensor.matmul(
                        ctx_ps[:, TB : 2 * TB],
                        v_sb[J][:, 65 * hB : 65 * hB + 65],
                        ex[:, TB : 2 * TB],
                        start=(J == 0),
                        stop=(J == NTT - 1),
                    )
                # transpose contraction dim must be a multiple of 32:
                # pad [65, .] to [96, .] with zeros in rows 65:96.
                ctx_sb = cs.tile([96, 2 * TB], F32R, tag="ctxsb")
                nc.vector.tensor_copy(ctx_sb[64:96, :], zc[64:96, :])
                nc.vector.tensor_copy(ctx_sb[0:65, :], ctx_ps)
                for q in range(4):  # 128-token output tiles within I
                    st = outp.tile([128, 128], F32, tag="st")
                    for h2 in range(2):
                        off = h2 * TB + q * 128
                        fin = ps_misc.tile([128, 96], F32R, tag="misc")
                        nc.tensor.transpose(
                            fin,
                            ctx_sb[:, off : off + 128],
                            identity[0:96, 0:96],
                        )
                        rec = tiny.tile([128, 1], F32, tag="rec")
                        nc.vector.reciprocal(rec, fin[:, 64:65])
                        nc.vector.tensor_scalar_mul(
                            st[:, 64 * h2 : 64 * h2 + 64], fin[:, 0:64], rec
                        )
                    nc.sync.dma_start(
                        out=out[ds(I * TB + q * 128, 128), ds(hp * 128, 128)],
                        in_=st,
                    )

            # k and v first (attention consumes both in full), then per
            # q-block start the attention units for that i-block.
            for T in range(NTB):
                xts = xpose_block(xk, T)
                proj_qk("k", xts, bkt, T)
            for T in range(NTB):
                xts = xpose_block(xv, T)
                for i in range(4):
                    tt = 4 * T + i
                    psv = ps_mm.tile([128, 2 * TB], F32, tag="mm")
                    for cc in range(NCC):
                        nc.tensor.matmul(
                            psv[:, 0:DH],
                            xts[cc][:, ts(i, 128)],
                            wts["v"][:, cc, :],
                            start=(cc == 0),
                            stop=(cc == NCC - 1),
                        )
                    for h in range(NHEAD):
                        nc.vector.tensor_add(
                            v_sb[tt][:, 65 * h : 65 * h + 64],
                            psv[:, 64 * h : 64 * h + 64],
                            bvb[:, 64 * h : 64 * h + 64],
                        )
            for T in range(NTB):
                xts = xpose_block(xq, T)
                proj_qk("q", xts, bqt, T)
            for T in range(NTB):
                for hp in range(NDD):
                    attention_unit(hp, I=T)
    nc.finalize()
    return nc


def _get_module():
    if "nc" not in _NC_CACHE:
        _NC_CACHE["nc"] = _build_module()
    return _NC_CACHE["nc"]


def kernel(**inputs) -> np.ndarray:
    from concourse.bass_utils import run_bass_kernel_spmd

    query = np.asarray(inputs["query"], dtype=np.float32)
    key = np.asarray(inputs["key"], dtype=np.float32)
    value = np.asarray(inputs["value"], dtype=np.float32)
    Wq = np.asarray(inputs["Wq"], dtype=np.float32)
    Wk = np.asarray(inputs["Wk"], dtype=np.float32)
    Wv = np.asarray(inputs["Wv"], dtype=np.float32)
    bq = np.asarray(inputs["bq"], dtype=np.float32)
    bk = np.asarray(inputs["bk"], dtype=np.float32)
    bv = np.asarray(inputs["bv"], dtype=np.float32)

    B = query.shape[0]
    nc = _get_module()
    in_maps = []
    for c in range(NCORES):
        b, g = c // 2, c % 2
        sl = slice(g * DH, (g + 1) * DH)
        in_maps.append(
            {
                "xq": np.ascontiguousarray(query[b]),
                "xk": np.ascontiguousarray(key[b]),
                "xv": np.ascontiguousarray(value[b]),
                "wq": np.ascontiguousarray(Wq[sl]),
                "wk": np.ascontiguousarray(Wk[sl]),
                "wv": np.ascontiguousarray(Wv[sl]),
                "bq": np.ascontiguousarray(bq[sl]),
                "bk": np.ascontiguousarray(bk[sl]),
                "bv": np.ascontiguousarray(bv[sl]),
            }
        )
    res = run_bass_kernel_spmd(nc, in_maps, core_ids=list(range(NCORES)))
    full = np.empty((B, S, C), dtype=np.float32)
    for c in range(NCORES):
        b, g = c // 2, c % 2
        full[b, :, g * DH : (g + 1) * DH] = res.results[c]["out"]
    return full


if __name__ == "__main__":
    import reference

    inputs = {k: np.asarray(v) for k, v in reference.setup_inputs().items()}
    got = kernel(**inputs)
    want = np.asarray(reference.reference(**reference.setup_inputs()))
    err = np.abs(got - want).max() / np.abs(want).max()
    print("rel err:", err)


# revision 29
# speedup vs baseline: 1.0399x; 1.0041x over previous
"""Trainium2 Bass kernel for a dense multi-head attention layer.

Problem (full shapes): query/key/value [4, 2048, 1024] fp32, Wq/Wk/Wv
[1024, 1024], bq/bk/bv [1024].  out = MHA(q,k,v) with H=16 heads of 64.

Sharding over 8 NeuronCores: core c handles batch c//2 and head-half c%2
(8 heads = 512 of the 1024 output dims).  Each core runs an identical
program on its shard; the host assembles the full [4, 2048, 1024] output.

Per-core pipeline (all matmuls in float32r = full-rate fp32 storage):
  1. PE-transpose W slices -> WT [c, d] and x tiles -> xT [c, t].
  2. Projections: kT/qT [d, t] (stored bf16; scores are scaled by 1/8 so
     bf16 rounding of q/k is harmless) and v [t, d] fp32 with a ones
     column appended per head (65-wide blocks).
  3. Attention per head-pair/i-block: scores computed transposed
     [j, i] = kT.T @ qT, exp on ScalarE directly from PSUM with the
     1/sqrt(64) scale folded in (no max-subtraction: scores are O(6)),
     then ctx.T [hd+1, i] = v_aug.T @ exp accumulated over j; row 64 is
     the softmax denominator for free.
  4. Finalize: PE-transpose [65, 128] chunks -> [t, hd+1], reciprocal of
     the denominator column, scale, DMA out.
"""

import numpy as np

_NC_CACHE = {}

S = 2048  # tokens per batch
C = 1024  # model dim (contraction)
DH = 512  # output dims per core (8 heads x 64)
NHEAD = 8  # heads per core
HD = 64  # head dim
NCORES = 8
TB = 512  # token block
NTB = S // TB  # 4
NCC = C // 128  # 8 c-chunks
NDD = DH // 128  # 4 d-tiles (= head pairs)
NTT = S // 128  # 16 t-tiles
SCALE = 0.125  # 1/sqrt(64)


def _build_module():
    import concourse.mybir as mybir
    import concourse.tile as tile
    from concourse import bacc
    from concourse.bass import ds, ts
    from concourse.masks import make_identity

    F32 = mybir.dt.float32
    F32R = mybir.dt.float32r
    BF16 = mybir.dt.bfloat16
    EXP = mybir.ActivationFunctionType.Exp

    nc = bacc.Bacc("TRN2", target_bir_lowering=False)

    xq = nc.dram_tensor("xq", [S, C], F32R, kind="ExternalInput")
    xk = nc.dram_tensor("xk", [S, C], F32R, kind="ExternalInput")
    xv = nc.dram_tensor("xv", [S, C], F32R, kind="ExternalInput")
    wq = nc.dram_tensor("wq", [DH, C], F32R, kind="ExternalInput")
    wk = nc.dram_tensor("wk", [DH, C], F32R, kind="ExternalInput")
    wv = nc.dram_tensor("wv", [DH, C], F32R, kind="ExternalInput")
    bq = nc.dram_tensor("bq", [DH], F32, kind="ExternalInput")
    bk = nc.dram_tensor("bk", [DH], F32, kind="ExternalInput")
    bv = nc.dram_tensor("bv", [DH], F32, kind="ExternalInput")
    out = nc.dram_tensor("out", [S, DH], F32, kind="ExternalOutput")

    with tile.TileContext(nc) as tc:
        from contextlib import ExitStack

        with ExitStack() as ctx:
            consts = ctx.enter_context(tc.tile_pool(name="consts", bufs=1))
            ld = ctx.enter_context(tc.tile_pool(name="ld", bufs=10))
            xt = ctx.enter_context(tc.tile_pool(name="xt", bufs=16))
            wt = ctx.enter_context(tc.tile_pool(name="wt", bufs=1))
            qk = ctx.enter_context(tc.tile_pool(name="qk", bufs=1))
            vp = ctx.enter_context(tc.tile_pool(name="vp", bufs=1))
            cs = ctx.enter_context(tc.tile_pool(name="cs", bufs=3))
            xp = ctx.enter_context(tc.tile_pool(name="xp", bufs=8))
            outp = ctx.enter_context(tc.tile_pool(name="outp", bufs=8))
            tiny = ctx.enter_context(tc.tile_pool(name="tiny", bufs=6))
            ps_mm = ctx.enter_context(
                tc.tile_pool(name="ps_mm", bufs=2, space="PSUM")
            )
            ps_ctx = ctx.enter_context(
                tc.tile_pool(name="ps_ctx", bufs=1, space="PSUM")
            )
            ps_misc = ctx.enter_context(
                tc.tile_pool(name="ps_misc", bufs=2, space="PSUM")
            )

            ident_f32 = consts.tile([128, 128], F32, tag="ident_f32")
            make_identity(nc, ident_f32)
            identity = consts.tile([128, 128], F32R, tag="ident")
            nc.vector.tensor_copy(identity, ident_f32)
            identity_bf = consts.tile([128, 128], BF16, tag="identb")
            nc.vector.tensor_copy(identity_bf, ident_f32)
            ones8 = consts.tile([128, 8], F32, tag="ones8")
            nc.vector.memset(ones8, 1.0)
            zc = consts.tile([128, 2 * TB], F32, tag="zc")
            nc.vector.memset(zc, 0.0)
            # Load the exp table during phase 1 (first ACTIVATE pays ~2.7us).
            warm = consts.tile([128, 1], F32, tag="warm")
            nc.scalar.activation(warm, ones8[:, 0:1], EXP)

            # Per-partition bias columns for qT/kT [d on partitions]:
            # col a holds bias[a*128 + p].
            bqt = consts.tile([128, NDD], F32, tag="bqt")
            nc.sync.dma_start(out=bqt, in_=bq[:].rearrange("(a p) -> p a", p=128))
            bkt = consts.tile([128, NDD], F32, tag="bkt")
            nc.sync.dma_start(out=bkt, in_=bk[:].rearrange("(a p) -> p a", p=128))
            # v bias broadcast along token partitions.
            bvb = consts.tile([128, DH], F32, tag="bvb")
            nc.sync.dma_start(
                out=bvb,
                in_=bv[:].rearrange("(o c) -> o c", o=1).to_broadcast([128, DH]),
            )

            # ---- weight transposes: W [d, c] -> WT [c, (cc, d)] ----
            # Cast fp32 -> bf16 during the load (SWDGE), then transpose
            # 128x128 blocks on the DMA xbar engine (HWDGE) -- zero PE work.
            wts = {}
            for name, wsrc in (("q", wq), ("k", wk), ("v", wv)):
                wtile = wt.tile([128, NCC, DH], BF16, tag=f"wt_{name}")
                wts[name] = wtile
                lds = []
                for p in range(NDD):
                    t = ld.tile([128, C], BF16, tag="ld")
                    nc.gpsimd.dma_start(out=t, in_=wsrc[ts(p, 128), :])
                    lds.append(t)
                for cc in range(NCC):
                    ps = ps_misc.tile([128, DH], BF16, tag="misc")
                    for p in range(NDD):
                        nc.tensor.transpose(
                            ps[:, ts(p, 128)],
                            lds[p][:, ts(cc, 128)],
                            identity_bf,
                        )
                    nc.vector.tensor_copy(wtile[:, cc, :], ps)

            # ---- projections, k and v first, then q ----
            # qkT[(name, dd, T)] : [128 d, 512 t] bf16
            qkt = {}
            # v_sb[tt] : [128 t, 8*65] fp32, ones in col 64 of each 65-block
            v_sb = []
            for tt in range(NTT):
                vt = vp.tile([128, NHEAD * 65], BF16, tag=f"v{tt}")
                v_sb.append(vt)
                ones_view = vt.rearrange("p (h c) -> p h c", c=65)[:, :, 64:65]
                nc.vector.tensor_copy(
                    ones_view, ones8.rearrange("p (h o) -> p h o", o=1)
                )

            def xpose_block(xsrc, T):
                """Load x[T*512:(T+1)*512] (cast to bf16) and return 8 xT
                tiles [128 c, 512 t] produced by xbar DMA transpose."""
                lds = []
                for i in range(4):
                    t = ld.tile([128, C], BF16, tag="ld")
                    nc.gpsimd.dma_start(out=t, in_=xsrc[ts(4 * T + i, 128), :])
                    lds.append(t)
                xts = []
                for cc in range(NCC):
                    ps = ps_misc.tile([128, TB], BF16, tag="misc")
                    for i in range(4):
                        nc.tensor.transpose(
                            ps[:, ts(i, 128)],
                            lds[i][:, ts(cc, 128)],
                            identity_bf,
                        )
                    xtile = xt.tile([128, TB], BF16, tag="xt")
                    nc.vector.tensor_copy(xtile, ps)
                    xts.append(xtile)
                return xts

            def proj_qk(name, xts, btile, T):
                for dd in range(NDD):
                    psq = ps_mm.tile([128, 2 * TB], F32, tag="mm")
                    for cc in range(NCC):
                        nc.tensor.matmul(
                            psq[:, 0:TB],
                            wts[name][:, cc, ts(dd, 128)],
                            xts[cc],
                            start=(cc == 0),
                            stop=(cc == NCC - 1),
                        )
                    qt = qk.tile([128, TB], BF16, tag=f"{name}{dd}{T}")
                    qkt[(name, dd, T)] = qt
                    nc.vector.tensor_scalar_add(
                        qt, psq[:, 0:TB], btile[:, dd : dd + 1]
                    )

            def attention_unit(hp, I):
                hA, hB = 2 * hp, 2 * hp + 1
                qtile = qkt[("q", hp, I)]
                ctx_ps = ps_ctx.tile([65, 2 * TB], F32, tag="ctx")
                for J in range(NTT):  # j tile of 128 keys
                    ktile = qkt[("k", hp, J // 4)]
                    jj = (J % 4) * 128
                    sc = ps_mm.tile([128, 2 * TB], F32, tag="mm")
                    nc.tensor.matmul(
                        sc[:, 0:TB],
                        ktile[0:64, jj : jj + 128],
                        qtile[0:64, :],
                    )
                    nc.tensor.matmul(
                        sc[:, TB : 2 * TB],
                        ktile[64:128, jj : jj + 128],
                        qtile[64:128, :],
                    )
                    ex = xp.tile([128, 2 * TB], BF16, tag="exp")
                    nc.scalar.activation(ex, sc, EXP, scale=SCALE)
                    nc.tensor.matmul(
                        ctx_ps[:, 0:TB],
                        v_sb[J][:, 65 * hA : 65 * hA + 65],
                        ex[:, 0:TB],
                        start=(J == 0),
                        stop=(J == NTT - 1),
                    )
                    nc.tensor.matmul(
                        ctx_ps[:, TB : 2 * TB],
                        v_sb[J][:, 65 * hB : 65 * hB + 65],
                        ex[:, TB : 2 * TB],
                        start=(J == 0),
                        stop=(J == NTT - 1),
                    )
                # transpose contraction dim must be a multiple of 32:
                # pad [65, .] to [96, .] with zeros in rows 65:96.
                ctx_sb = cs.tile([96, 2 * TB], F32R, tag="ctxsb")
                nc.vector.tensor_copy(ctx_sb[64:96, :], zc[64:96, :])
                nc.vector.tensor_copy(ctx_sb[0:65, :], ctx_ps)
                for q in range(4):  # 128-token output tiles within I
                    st = outp.tile([128, 128], F32, tag="st")
                    for h2 in range(2):
                        off = h2 * TB + q * 128
                        fin = ps_misc.tile([128, 96], F32R, tag="misc")
                        nc.tensor.transpose(
                            fin,
                            ctx_sb[:, off : off + 128],
                            identity[0:96, 0:96],
                        )
                        rec = tiny.tile([128, 1], F32, tag="rec")
                        nc.vector.reciprocal(rec, fin[:, 64:65])
                        nc.vector.tensor_scalar_mul(
                            st[:, 64 * h2 : 64 * h2 + 64], fin[:, 0:64], rec
                        )
                    nc.sync.dma_start(
                        out=out[ds(I * TB + q * 128, 128), ds(hp * 128, 128)],
                        in_=st,
                    )

            # k and v first (attention consumes both in full), then per
            # q-block start the attention units for that i-block.
            for T in range(NTB):
                xts = xpose_block(xk, T)
                proj_qk("k", xts, bkt, T)
            for T in range(NTB):
                xts = xpose_block(xv, T)
                for i in range(4):
                    tt = 4 * T + i
                    psv = ps_mm.tile([128, 2 * TB], F32, tag="mm")
                    for cc in range(NCC):
                        nc.tensor.matmul(
                            psv[:, 0:DH],
                            xts[cc][:, ts(i, 128)],
                            wts["v"][:, cc, :],
                            start=(cc == 0),
                            stop=(cc == NCC - 1),
                        )
                    for h in range(NHEAD):
                        nc.vector.tensor_add(
                            v_sb[tt][:, 65 * h : 65 * h + 64],
                            psv[:, 64 * h : 64 * h + 64],
                            bvb[:, 64 * h : 64 * h + 64],
                        )
            for T in range(NTB):
                xts = xpose_block(xq, T)
                proj_qk("q", xts, bqt, T)
            for T in range(NTB):
                for hp in range(NDD):
                    attention_unit(hp, I=T)
    nc.finalize()
    return nc


def _get_module():
    if "nc" not in _NC_CACHE:
        _NC_CACHE["nc"] = _build_module()
    return _NC_CACHE["nc"]


def kernel(**inputs) -> np.ndarray:
    from concourse.bass_utils import run_bass_kernel_spmd

    query = np.asarray(inputs["query"], dtype=np.float32)
    key = np.asarray(inputs["key"], dtype=np.float32)
    value = np.asarray(inputs["value"], dtype=np.float32)
    Wq = np.asarray(inputs["Wq"], dtype=np.float32)
    Wk = np.asarray(inputs["Wk"], dtype=np.float32)
    Wv = np.asarray(inputs["Wv"], dtype=np.float32)
    bq = np.asarray(inputs["bq"], dtype=np.float32)
    bk = np.asarray(inputs["bk"], dtype=np.float32)
    bv = np.asarray(inputs["bv"], dtype=np.float32)

    B = query.shape[0]
    nc = _get_module()
    in_maps = []
    for c in range(NCORES):
        b, g = c // 2, c % 2
        sl = slice(g * DH, (g + 1) * DH)
        in_maps.append(
            {
                "xq": np.ascontiguousarray(query[b]),
                "xk": np.ascontiguousarray(key[b]),
                "xv": np.ascontiguousarray(value[b]),
                "wq": np.ascontiguousarray(Wq[sl]),
                "wk": np.ascontiguousarray(Wk[sl]),
                "wv": np.ascontiguousarray(Wv[sl]),
                "bq": np.ascontiguousarray(bq[sl]),
                "bk": np.ascontiguousarray(bk[sl]),
                "bv": np.ascontiguousarray(bv[sl]),
            }
        )
    res = run_bass_kernel_spmd(nc, in_maps, core_ids=list(range(NCORES)))
    full = np.empty((B, S, C), dtype=np.float32)
    for c in range(NCORES):
        b, g = c // 2, c % 2
        full[b, :, g * DH : (g + 1) * DH] = res.results[c]["out"]
    return full


if __name__ == "__main__":
    import reference

    inputs = {k: np.asarray(v) for k, v in reference.setup_inputs().items()}
    got = kernel(**inputs)
    want = np.asarray(reference.reference(**reference.setup_inputs()))
    err = np.abs(got - want).max() / np.abs(want).max()
    print("rel err:", err)
